# revision 7
# baseline (speedup 1.0000x reference)
"""BalancedMoE (B=8192, D=2048, E=8, top-2) on 8 Trainium2 NeuronCores.

Strategy: expert-parallel with host-side sparse dispatch + 2-weight-set
load balancing.
  - Host computes gate logits / top-2 routing / softmax gates, gathers
    each expert's tokens and transposes them into [D, C] so the device
    needs no on-chip transposes.
  - Plain expert-parallel pads every core to the BIGGEST expert's token
    count (C_max=2234 here vs mean 2048). Instead each core holds TWO
    expert weight sets in SBUF and processes two token slots (a, b); a
    small host-side solver splits oversized experts across cores so the
    per-core column count drops to ~max(C_max/2 rounded pairings)
    (2113 here) — a ~5% compute-floor cut.
  - bf16 operands/outputs: matmul rate is identical to fp32r (1 col/
    cycle) but halves every DMA stream; rel-err ~2.7e-3 vs the 2e-2 gate.
  - Host scatters the per-(core,slot) outputs back and combines with
    the gate weights.

Per-core Bass kernel: outT[o, t] = sum_d W_s[o, d] * toks[t, d] + b_s[o]
with s = the slot (weight set) the column t belongs to.
  lhsT = W_s^T tiles (stationary), rhs = toksT tiles (moving).
"""

import os

import numpy as np

P = 128
B = 8192
D_LAT = 1024
D_EMB = 1024
D = D_LAT + D_EMB  # 2048
E = 8
TOPK = 2
N_CORES = 8


# ----------------------------------------------------------------- device ---

_cache = {}


def _ntff_shim():
    """Register the axon NTFF profile hook that the boot skips when
    antenv.axon_hooks is missing (so BASS_TRACE=1 yields exec_time_ns)."""
    import sys
    import types

    if "antenv.axon_hooks" in sys.modules:
        return
    holder = [None]
    mod = types.ModuleType("antenv.axon_hooks")
    mod.set_axon_ntff_profile_hook = lambda h: holder.__setitem__(0, h)
    mod.get_axon_ntff_profile_hook = lambda: holder[0]
    sys.modules["antenv.axon_hooks"] = mod
    try:
        import antenv

        antenv.axon_hooks = mod
        from trn_agent_boot.trn_boot import _ntff_profile_via_ctypes

        mod.set_axon_ntff_profile_hook(
            _ntff_profile_via_ctypes("/opt/axon/libaxon_pjrt.so")
        )
    except Exception:
        pass


def _split_slot(C, lead_small=False, tail_small=False):
    """Tile widths for one slot. bf16 runs at 1 col/cycle at any width.
    lead_small: 256-col first tile so the first matmuls need only a small
    token download (startup is DMA-latency-bound).
    tail_small: keep the final tile <= 256 so the end-of-kernel drain
    (last vector op + last output DMA) is short."""
    tiles = []
    if lead_small and C >= 768:
        tiles.append(256)
        C -= 256
    while C > 512:
        tiles.append(512)
        C -= 512
    if tail_small and C > 256:
        tiles.extend([C - 186, 186])
    elif C:
        tiles.append(C)
    return tiles


# set-0 weight m-chunk batches, split across BOTH HWDGE queues (Act + SP)
# so the stream runs at ~2x one queue's bandwidth: fine-grained early
# (compute consumes a chunk every ~1.7us), coarse later (fewer triggers).
_W_ACT = [(0, 1), (2, 3), (4, 6), (8, 10), (12, 14)]
_W_SP = [(1, 2), (3, 4), (6, 8), (10, 12), (14, 16)]
# set-1 batches all ride Act (idle after ~35us; set 1 isn't needed until
# the slot-b tiles start at ~130us)
_W_SET1 = [(0, 4), (4, 8), (8, 12), (12, 16)]


def _build(Ca, Cb, dt_name):
    import concourse.mybir as mybir
    from concourse import bacc
    from concourse.bass import ds
    from concourse.tile import TileContext

    dt_in = getattr(mybir.dt, dt_name)
    dt_out = mybir.dt.bfloat16 if dt_name == "bfloat16" else mybir.dt.float32
    KT = D // P
    MT = D // P
    C = Ca + Cb
    # (size, weight-set) per moving tile; slot-a tiles first so set 1 isn't
    # needed until ~half way through the kernel (its DMA has ~100us of slack)
    tiles = [(sz, 0) for sz in _split_slot(Ca, lead_small=True)]
    tiles += [(sz, 1) for sz in _split_slot(Cb, tail_small=True)]
    nsets = 2 if Cb else 1

    nc = bacc.Bacc(
        "TRN2", target_bir_lowering=False, debug=False, num_devices=N_CORES
    )
    # wp[s, m, ki, ko, o] = W_set_s[m*128 + o, ko*128 + ki] — per-m chunks
    # are contiguous so each weight-chunk DMA moves 4KB/partition runs.
    wp = nc.dram_tensor("wp", [nsets, MT, P, KT, P], dt_in, kind="ExternalInput")
    toksT = nc.dram_tensor("toksT", [D, C], dt_in, kind="ExternalInput")
    bias = nc.dram_tensor("bias", [nsets, D], mybir.dt.float32, kind="ExternalInput")
    outT = nc.dram_tensor("outT", [D, C], dt_out, kind="ExternalOutput")

    t_r = toksT.ap().rearrange("(ko ki) n -> ki ko n", ki=P)
    o_r = outT.ap().rearrange("(mo mi) n -> mi mo n", mi=P)
    b_r = bias.ap().rearrange("s (mo mi) -> mi s mo", mi=P)
    # partition-major view of the weight chunks so a multi-chunk DMA lands
    # in [P, m, KT, P] SBUF order
    w_r = wp.ap().rearrange("s m ki ko o -> ki s m ko o")

    with TileContext(nc) as tc:
        with (
            tc.tile_pool(name="w", bufs=1) as w_pool,
            tc.tile_pool(name="tok", bufs=4) as tok_pool,
            tc.tile_pool(name="out", bufs=3) as out_pool,
            tc.tile_pool(name="bias", bufs=1) as b_pool,
            tc.tile_pool(name="ps", bufs=8, space="PSUM") as ps_pool,
        ):
            bias_tile = b_pool.tile([P, nsets, MT], mybir.dt.float32)
            nc.sync.dma_start(bias_tile[:], b_r)
            tok_tiles = {}

            def load_toks(n, n_off, n_sz, chunks=None):
                t_full = tok_pool.tile([P, KT, 512], dt_in, tag="tok")
                t_tile = t_full[:, :, :n_sz]
                if chunks:
                    # k-slice chunks so the first matmuls only wait for the
                    # slices they read — but few enough triggers that the SP
                    # sequencer (~0.6us per DMA trigger) isn't the start gate
                    k = 0
                    for w in chunks:
                        nc.sync.dma_start(
                            t_tile[:, k : k + w, :],
                            t_r[:, k : k + w, ds(n_off, n_sz)],
                        )
                        k += w
                else:
                    nc.sync.dma_start(t_tile, t_r[:, :, ds(n_off, n_sz)])
                tok_tiles[n] = t_tile

            w_tiles = [[None] * MT for _ in range(nsets)]

            def load_w_batch(s, lo, hi, eng):
                w_t = w_pool.tile([P, hi - lo, KT, P], dt_in, tag=f"w{s}_{lo}")
                eng.dma_start(w_t[:], w_r[:, s, lo:hi])
                for m in range(lo, hi):
                    w_tiles[s][m] = w_t[:, m - lo]

            # issue order ~= consumption order. Act queue: w0-batch, other
            # even set-0 batches, then all of set 1. SP queue: first token
            # tile (k-chunked, smallest chunk first so matmul 0 starts
            # ASAP), odd set-0 batches, then (from the compute loop) the
            # n=1 token prefetch and the output stream.
            load_w_batch(0, *_W_ACT[0], nc.scalar)
            load_toks(0, 0, tiles[0][0], chunks=(4, 12))
            for j in range(len(_W_SP)):
                load_w_batch(0, *_W_SP[j], nc.sync)
                if j + 1 < len(_W_ACT):
                    load_w_batch(0, *_W_ACT[j + 1], nc.scalar)
            if nsets > 1:
                for lo, hi in _W_SET1:
                    load_w_batch(1, lo, hi, nc.scalar)

            n_offs = []
            off = 0
            for sz, _s in tiles:
                n_offs.append(off)
                off += sz

            for n, (n_sz, s) in enumerate(tiles):
                # prefetch two tiles ahead, triggered BEFORE this iteration's
                # output-DMA chain hits the SP queue (the out-DMA triggers
                # wait on vector ops, head-of-line blocking later triggers)
                if n >= 1:
                    for tgt in (n + 1, n + 2):
                        if tgt < len(tiles) and tgt not in tok_tiles:
                            load_toks(tgt, n_offs[tgt], tiles[tgt][0])
                t_tile = tok_tiles.pop(n)
                n_off = n_offs[n]
                for m in range(MT):
                    # issue the n=1 token prefetch from the middle of n=0's
                    # SP stream: early enough to land before n=1 starts,
                    # late enough not to steal HBM from the weight stream
                    if n == 0 and m == 8 and len(tiles) > 1:
                        load_toks(1, n_offs[1], tiles[1][0])
                    ps_full = ps_pool.tile([P, 512], mybir.dt.float32, tag="ps")
                    ps = ps_full[:, :n_sz]
                    for k in range(KT):
                        nc.tensor.matmul(
                            ps,
                            w_tiles[s][m][:, k, :],
                            t_tile[:, k, :],
                            start=(k == 0),
                            stop=(k == KT - 1),
                        )
                    o_full = out_pool.tile([P, 512], dt_out, tag="out")
                    o_tile = o_full[:, :n_sz]
                    nc.vector.tensor_scalar_add(
                        o_tile, ps, bias_tile[:, s, m : m + 1]
                    )
                    nc.sync.dma_start(o_r[:, m, ds(n_off, n_sz)], o_tile)
    nc.compile()
    return nc


def _get_program(Ca, Cb, dt_name):
    key = (Ca, Cb, dt_name)
    if key not in _cache:
        _cache[key] = _build(Ca, Cb, dt_name)
    return _cache[key]


# ------------------------------------------------------------- host: pack ---


def _solve_pack(counts):
    """2-weight-set slot packing: each core gets slot_a (Ca cols, weight
    set 0) and slot_b (Cb cols, weight set 1); every slot holds a chunk of
    ONE expert's tokens. x experts split across two a-slots, 8-2x experts
    use one core's (a,b), x experts split across two b-slots. Minimizes
    N = Ca + Cb (the per-core padded column count)."""
    n = len(counts)
    order = np.argsort(-np.asarray(counts), kind="stable")
    c = [int(counts[i]) for i in order]
    best = None
    for x in range(0, n // 2 + 1):
        if x == 0:
            a, b, N = c[0], 0, c[0]
        else:
            a = (c[0] + 1) // 2
            b = (c[n - x] + 1) // 2
            mids = c[x : n - x]
            N = max(a + b, mids[0] if mids else 0)
            a = max(a, N - b)
            b = N - a
        if best is None or N < best[0]:
            best = (N, x, a, b)
    N, x, a, b = best
    plan = [[] for _ in range(n)]  # per core: list of (slot, expert, lo, hi)
    for j in range(x):  # biggest experts -> a-slots of cores 2j, 2j+1
        e = int(order[j])
        ce = c[j]
        cut = min(a, ce)
        plan[2 * j].append(("a", e, 0, cut))
        if ce > cut:
            plan[2 * j + 1].append(("a", e, cut, ce))
    for k, j in enumerate(range(x, n - x)):  # mids -> (a,b) of one core
        e = int(order[j])
        ce = c[j]
        core = 2 * x + k
        cut = min(a, ce)
        plan[core].append(("a", e, 0, cut))
        if ce > cut:
            plan[core].append(("b", e, cut, ce))
    for j in range(x):  # smallest experts -> b-slots of cores 2j, 2j+1
        e = int(order[n - x + j])
        ce = c[n - x + j]
        cut = min(b, ce)
        plan[2 * j].append(("b", e, 0, cut))
        if ce > cut:
            plan[2 * j + 1].append(("b", e, cut, ce))
    # sanity: coverage and capacity
    cov = [0] * n
    for core, items in enumerate(plan):
        used = {"a": 0, "b": 0}
        owners = {"a": set(), "b": set()}
        for slot, e, lo, hi in items:
            used[slot] += hi - lo
            owners[slot].add(e)
            cov[e] += hi - lo
        if used["a"] > a or used["b"] > b:
            return None
        if len(owners["a"]) > 1 or len(owners["b"]) > 1:
            return None
    if cov != [int(v) for v in counts]:
        return None
    return N, x, a, b, plan


# ------------------------------------------------------------------- host ---


def kernel(x, y, W_experts, b_experts, W_gate, b_gate):
    x = np.asarray(x, dtype=np.float32)
    y = np.asarray(y, dtype=np.float32)
    W_experts = np.asarray(W_experts, dtype=np.float32)
    b_experts = np.asarray(b_experts, dtype=np.float32)
    W_gate = np.asarray(W_gate, dtype=np.float32)
    b_gate = np.asarray(b_gate, dtype=np.float32)

    inp = np.concatenate([x, y], axis=1)  # [B, D]

    # ---- routing (host) ----
    logits = inp.astype(np.float64) @ W_gate.T.astype(np.float64) + b_gate
    order = np.argsort(-logits, axis=1, kind="stable")
    top2 = order[:, :TOPK]  # [B, 2]
    v = np.take_along_axis(logits, top2, axis=1)
    v = v - v.max(axis=1, keepdims=True)
    ev = np.exp(v)
    g = (ev / ev.sum(axis=1, keepdims=True)).astype(np.float32)  # [B, 2]

    counts = np.bincount(top2.ravel(), minlength=E)

    idx_list = []
    wgt_list = []
    for e in range(E):
        m0 = top2[:, 0] == e
        m1 = top2[:, 1] == e
        idx_e = np.concatenate([np.nonzero(m0)[0], np.nonzero(m1)[0]])
        w_e = np.concatenate([g[m0, 0], g[m1, 1]])
        idx_list.append(idx_e)
        wgt_list.append(w_e)

    dt_name = os.environ.get("MOE_DT", "bfloat16")
    if dt_name == "bfloat16":
        import ml_dtypes

        np_in_dt = np.dtype(ml_dtypes.bfloat16)
    else:
        np_in_dt = np.dtype(np.float32)

    pack = None
    if os.environ.get("MOE_PACK", "1") == "1" and E == N_CORES:
        pack = _solve_pack(counts)
    if pack is not None and pack[3] > 0:
        N, _x, Ca, Cb, plan = pack
        Ca = max(Ca, 512)  # tiling floor
    else:
        Ca, Cb = max(512, int(counts.max())), 0
        plan = [[("a", e, 0, int(counts[e]))] for e in range(E)]

    inpT = np.ascontiguousarray(inp.T.astype(np_in_dt))  # [D, B]
    MT = KT = D // P
    wpacked = [None] * E

    def _wpack(e):
        if wpacked[e] is None:
            # wp[m, ki, ko, o] = W_e[m*128 + o, ko*128 + ki]
            wpacked[e] = np.ascontiguousarray(
                W_experts[e]
                .reshape(MT, P, KT, P)
                .transpose(0, 3, 2, 1)
                .astype(np_in_dt)
            )
        return wpacked[e]

    nsets = 2 if Cb else 1
    slot_off = {"a": 0, "b": Ca}
    slot_idx = {"a": 0, "b": 1}
    in_maps = []
    for core in range(E):
        toksT = np.zeros((D, Ca + Cb), dtype=np_in_dt)
        wp = np.zeros((nsets, MT, P, KT, P), dtype=np_in_dt)
        bias = np.zeros((nsets, D), dtype=np.float32)
        for slot, e, lo, hi in plan[core]:
            off = slot_off[slot]
            toksT[:, off : off + (hi - lo)] = inpT[:, idx_list[e][lo:hi]]
            wp[slot_idx[slot]] = _wpack(e)
            bias[slot_idx[slot]] = b_experts[e]
        in_maps.append({"wp": wp, "toksT": toksT, "bias": bias})

    # ---- device ----
    if os.environ.get("BASS_TRACE"):
        _ntff_shim()
    from concourse.bass_utils import run_bass_kernel_spmd

    nc = _get_program(Ca, Cb, dt_name)
    res = None
    for attempt in range(3):
        try:
            res = run_bass_kernel_spmd(nc, in_maps, core_ids=list(range(N_CORES)))
            break
        except Exception:
            # the axon-tunneled device occasionally reports a transient
            # NRT_EXEC_UNIT_UNRECOVERABLE; it recovers after a short wait
            if attempt == 2:
                raise
            import time

            time.sleep(20 * (attempt + 1))
            try:
                import jax

                jax.clear_caches()
            except Exception:
                pass
    globals()["_last_res"] = res
    if res.exec_time_ns is not None:
        print(f"HW exec time: {res.exec_time_ns} ns")

    # ---- combine (host) ----
    fused = np.zeros((B, D), dtype=np.float32)
    for core in range(E):
        outT = res.results[core]["outT"]
        for slot, e, lo, hi in plan[core]:
            off = slot_off[slot]
            rows = outT[:, off : off + (hi - lo)].T.astype(np.float32)
            fused[idx_list[e][lo:hi]] += rows * wgt_list[e][lo:hi, None]
    return fused
